# revision 3
# baseline (speedup 1.0000x reference)
"""CLIP-style contrastive (NT-Xent) loss on 8 Trainium2 NeuronCores.

v2: quadrant-band symmetric scheme — exploits sim-matrix symmetry to halve
the ScalarE exp work (the v1 bottleneck: 67us of EXP at full width).

  - Data-parallel batch shard (512 rows/core), towers computed in transposed
    activation layout exactly as v1 (weights as stationary lhsT, bf16).
  - Normalized projections AllGather'd per modality (img AG overlaps the txt
    tower).  Each core builds zf[mod] = [128, 4096+2304] bf16: the gathered
    z plus a wrap-around extension so every cyclic band is contiguous.
  - Similarity quadrants: img*img and txt*txt are symmetric -> each 128-row
    chunk computes only a cyclic band of W=2176 columns (tiles u=0..17; u=0
    strict-upper masked, u=17 lower-incl-diag masked).  Row sums come from a
    DVE reduce over the exp'd band; the mirror half of each row sum is
    recovered from column sums (tiles u=0..14 full + u=15 strict-lower
    masked).  img*txt is computed once (img rows x all txt cols): row sums
    feed img rows, column sums feed txt rows.  Verified exact in fp64
    (proto_quad.py).
  - Per-core band offsets (c*512) enter through ONE runtime register
    (reg_load from an int32 input) used as a dynamic ds() offset on the
    moving operand; everything else is SPMD-uniform.
  - Column sums are ones-matmuls whose one-hot selector lhsT places each
    512-col piece on its own PSUM partition, so a single wide DVE copy
    extracts a whole row-chunk's colsums (DMA cannot read PSUM).
  - Host finish (fp64): T'_g = rowsum_g + colsum_g + exp(pos_g/t);
    loss = mean(log T' - pos/t).  No giant diag cancellation: the diagonal
    is excluded on-device by the masks.
"""

import os

os.environ.setdefault("NEURON_RT_DBG_RDH_CC", "0")

import numpy as np
import ml_dtypes

import concourse.bacc as bacc
import concourse.bass as bass
import concourse.mybir as mybir
import concourse.tile as tile
from concourse.bass import ds
from concourse.bass_utils import run_bass_kernel_spmd

NCORES = 8
B, DIN, DE, DH, DP = 4096, 1024, 512, 256, 128
S = B // NCORES            # 512 per-core batch shard
TS = 128
NBT = B // TS              # 32 column tiles per modality
W_II = 17                  # band tiles u=0..W_II (edges masked)
U_B = NBT - W_II           # 15: colsum tiles u=0..U_B (u=U_B strict-lower)
BAND = (W_II + 1) * TS     # 2304 band cols per row-chunk
CSPAN = (U_B + 1) * TS     # 2048 colsum span per band row-chunk
ZEXT = B + BAND            # 6400 zf cols incl wrap extension
TEMP = 0.07
INV_T = 1.0 / TEMP
NEG = -1.0e30

F32 = mybir.dt.float32
BF16 = mybir.dt.bfloat16
I32 = mybir.dt.int32
NPBF = ml_dtypes.bfloat16

# band chunks: (esc offset, width); chunk 0 holds edge u=0, chunk 2 edge u=17
BCHUNKS = ((0, 1024), (1024, 1024), (2048, 256))

_CACHE: dict = {}


def _build():
    nc = bacc.Bacc("TRN2", target_bir_lowering=False, debug=False,
                   num_devices=NCORES)

    t_in = {}
    for m in ("img", "txt"):
        t_in[f"xT_{m}"] = nc.dram_tensor(f"xT_{m}", [DIN, S], BF16,
                                         kind="ExternalInput")
        t_in[f"We_{m}"] = nc.dram_tensor(f"We_{m}", [DIN, DE], BF16,
                                         kind="ExternalInput")
        t_in[f"Wp1_{m}"] = nc.dram_tensor(f"Wp1_{m}", [DE, DH], BF16,
                                          kind="ExternalInput")
        t_in[f"Wp2_{m}"] = nc.dram_tensor(f"Wp2_{m}", [DH, DP], BF16,
                                          kind="ExternalInput")
        t_in[f"beT_{m}"] = nc.dram_tensor(f"beT_{m}", [128, DE // 128], F32,
                                          kind="ExternalInput")
        t_in[f"bp1T_{m}"] = nc.dram_tensor(f"bp1T_{m}", [128, DH // 128], F32,
                                           kind="ExternalInput")
        t_in[f"bp2T_{m}"] = nc.dram_tensor(f"bp2T_{m}", [128, DP // 128], F32,
                                           kind="ExternalInput")
    t_in["masks"] = nc.dram_tensor("masks", [128, 2 * TS], F32,
                                   kind="ExternalInput")
    t_in["cmask"] = nc.dram_tensor("cmask", [128, TS], BF16,
                                   kind="ExternalInput")
    t_in["sel"] = nc.dram_tensor("sel", [128, 16 * 16], BF16,
                                 kind="ExternalInput")
    t_in["voff"] = nc.dram_tensor("voff", [1, 1], I32, kind="ExternalInput")

    t_out = {
        "rows": nc.dram_tensor("rows", [128, 8], F32, kind="ExternalOutput"),
        "pos": nc.dram_tensor("pos", [1, S], F32, kind="ExternalOutput"),
        "colII": nc.dram_tensor("colII", [16, 512], F32,
                                kind="ExternalOutput"),
        "colIT": nc.dram_tensor("colIT", [32, 512], F32,
                                kind="ExternalOutput"),
        "colTT": nc.dram_tensor("colTT", [16, 512], F32,
                                kind="ExternalOutput"),
    }

    with tile.TileContext(nc) as tc:
        _emit(nc, tc, t_in, t_out)
    nc.compile()
    return nc


def _load_weights(nc, wpool, t_in, m):
    xt = wpool.tile([128, (DIN // 128) * S], BF16, name=f"xt_{m}")
    we = wpool.tile([128, (DIN // 128) * DE], BF16, name=f"we_{m}")
    for k in range(DIN // 128):
        nc.sync.dma_start(out=we[:, k * DE:(k + 1) * DE],
                          in_=t_in[f"We_{m}"][128 * k:128 * (k + 1), :])
        nc.sync.dma_start(out=xt[:, k * S:(k + 1) * S],
                          in_=t_in[f"xT_{m}"][128 * k:128 * (k + 1), :])
    wp1 = wpool.tile([128, (DE // 128) * DH], BF16, name=f"wp1_{m}")
    for k in range(DE // 128):
        nc.sync.dma_start(out=wp1[:, k * DH:(k + 1) * DH],
                          in_=t_in[f"Wp1_{m}"][128 * k:128 * (k + 1), :])
    wp2 = wpool.tile([128, (DH // 128) * DP], BF16, name=f"wp2_{m}")
    for k in range(DH // 128):
        nc.sync.dma_start(out=wp2[:, k * DP:(k + 1) * DP],
                          in_=t_in[f"Wp2_{m}"][128 * k:128 * (k + 1), :])
    beT = wpool.tile([128, DE // 128], F32, name=f"beT_{m}")
    nc.sync.dma_start(out=beT[:], in_=t_in[f"beT_{m}"][:, :])
    bp1T = wpool.tile([128, DH // 128], F32, name=f"bp1T_{m}")
    nc.sync.dma_start(out=bp1T[:], in_=t_in[f"bp1T_{m}"][:, :])
    bp2T = wpool.tile([128, DP // 128], F32, name=f"bp2T_{m}")
    nc.sync.dma_start(out=bp2T[:], in_=t_in[f"bp2T_{m}"][:, :])
    return dict(xt=xt, we=we, wp1=wp1, wp2=wp2, beT=beT, bp1T=bp1T, bp2T=bp2T)


def _project_normalize(nc, pps, psb, apool, w, m, ones_col, ones_row):
    """One tower in transposed layout; returns (zn fp32, znb bf16)."""
    Exp = mybir.ActivationFunctionType.Exp
    Ln = mybir.ActivationFunctionType.Ln
    add = mybir.AluOpType.add
    mx = mybir.AluOpType.max

    h = psb.tile([128, (DE // 128) * S], BF16, tag="h")
    for mm in range(DE // 128):
        ph = pps.tile([128, S], F32, tag="simps")
        for k in range(DIN // 128):
            nc.tensor.matmul(
                ph[:],
                w["we"][:, k * DE + 128 * mm: k * DE + 128 * (mm + 1)],
                w["xt"][:, k * S:(k + 1) * S],
                start=(k == 0), stop=(k == DIN // 128 - 1))
        nc.vector.tensor_scalar(
            out=h[:, mm * S:(mm + 1) * S], in0=ph[:],
            scalar1=w["beT"][:, mm:mm + 1], scalar2=None, op0=add)
    g = psb.tile([128, (DH // 128) * S], BF16, tag="g")
    for mm in range(DH // 128):
        pg = pps.tile([128, S], F32, tag="simps")
        for k in range(DE // 128):
            nc.tensor.matmul(
                pg[:],
                w["wp1"][:, k * DH + 128 * mm: k * DH + 128 * (mm + 1)],
                h[:, k * S:(k + 1) * S],
                start=(k == 0), stop=(k == DE // 128 - 1))
        nc.vector.tensor_scalar(
            out=g[:, mm * S:(mm + 1) * S], in0=pg[:],
            scalar1=w["bp1T"][:, mm:mm + 1], scalar2=0.0, op0=add, op1=mx)
    pz = pps.tile([128, S], F32, tag="simps")
    for k in range(DH // 128):
        nc.tensor.matmul(pz[:], w["wp2"][:, k * DP: k * DP + 128],
                         g[:, k * S:(k + 1) * S],
                         start=(k == 0), stop=(k == DH // 128 - 1))
    z = psb.tile([128, S], F32, tag=f"z_{m}")
    nc.vector.tensor_scalar(out=z[:], in0=pz[:], scalar1=w["bp2T"][:, 0:1],
                            scalar2=None, op0=add)

    # normalize columns: inv = exp(-0.5 * ln(sum z^2)) per batch column
    sq = psb.tile([128, S], F32, tag="sq")
    nc.vector.tensor_mul(sq[:], z[:], z[:])
    pssq = pps.tile([16, S], F32, tag="colps")
    nc.tensor.matmul(pssq[0:1, :], ones_col[:], sq[:], start=True, stop=True)
    lnr = psb.tile([1, S], F32, tag="lnr")
    nc.scalar.activation(lnr[:], pssq[0:1, :], Ln)
    inv = psb.tile([1, S], F32, tag="inv")
    nc.scalar.activation(inv[:], lnr[:], Exp, scale=-0.5)
    pinvb = pps.tile([128, S], F32, tag="simps")
    nc.tensor.matmul(pinvb[:], ones_row[:], inv[:], start=True, stop=True)
    zn = apool.tile([128, S], F32, name=f"zn_{m}")
    nc.vector.tensor_mul(zn[:], z[:], pinvb[:])
    znb = apool.tile([128, S], BF16, name=f"znb_{m}")
    nc.vector.tensor_copy(znb[:], zn[:])
    return zn, znb


def _emit(nc, tc, t_in, t_out):
    Exp = mybir.ActivationFunctionType.Exp
    add = mybir.AluOpType.add
    mult = mybir.AluOpType.mult

    with tc.tile_pool(name="const", bufs=1) as cpool, \
         tc.tile_pool(name="wpool", bufs=1) as wpool, \
         tc.tile_pool(name="actpool", bufs=1) as apool, \
         tc.tile_pool(name="projsb", bufs=2) as psb, \
         tc.tile_pool(name="escp", bufs=2) as escp, \
         tc.tile_pool(name="psum", bufs=2, space="PSUM") as pps, \
         tc.tile_pool(name="dram", bufs=1, space="DRAM") as dram:

        # --- tiny constants / per-core offset register -------------------
        vofft = wpool.tile([1, 1], I32, name="vofft")
        nc.sync.dma_start(out=vofft[:], in_=t_in["voff"][:, :])
        masks = wpool.tile([128, 2 * TS], F32, name="masks")
        nc.sync.dma_start(out=masks[:], in_=t_in["masks"][:, :])
        cmask = wpool.tile([128, TS], BF16, name="cmask")
        nc.sync.dma_start(out=cmask[:], in_=t_in["cmask"][:, :])
        sel = wpool.tile([128, 16 * 16], BF16, name="sel")
        nc.sync.dma_start(out=sel[:], in_=t_in["sel"][:, :])

        reg = nc.tensor.alloc_register("voff_reg")
        nc.reg_load(reg, vofft[0:1, 0:1])
        v = nc.snap(reg, min_val=0, max_val=(NCORES - 1) * S)

        ones_col = cpool.tile([128, 1], F32)
        nc.any.memset(ones_col[:], 1.0)
        ones_row = cpool.tile([1, 128], F32)
        nc.any.memset(ones_row[:], 1.0)
        onesb = cpool.tile([128, 1], BF16)
        nc.any.memset(onesb[:], 1.0)

        # --- tower operands (img first: its projection starts first) -----
        w_all = {m: _load_weights(nc, wpool, t_in, m) for m in ("img", "txt")}

        # --- towers + AllGathers -----------------------------------------
        zn, znb, zf = {}, {}, {}
        for m in ("img", "txt"):
            zf[m] = apool.tile([128, ZEXT], BF16, name=f"zf_{m}")
        for m in ("img", "txt"):
            zn[m], znb[m] = _project_normalize(
                nc, pps, psb, apool, w_all[m], m, ones_col, ones_row)
            cc_in = dram.tile([128, S], BF16, name=f"cc_in_{m}")
            nc.scalar.dma_start(out=cc_in[:, :], in_=znb[m][:])
            cc_o = dram.tile([128 * NCORES, S], BF16, name=f"cc_out_{m}",
                             addr_space="Shared")
            nc.gpsimd.collective_compute(
                "AllGather", mybir.AluOpType.bypass,
                replica_groups=[list(range(NCORES))],
                ins=[cc_in[:]], outs=[cc_o[:]])
            for j in range(NCORES):
                nc.sync.dma_start(
                    out=zf[m][:, S * j:S * (j + 1)],
                    in_=cc_o[128 * j:128 * (j + 1), :])
            # wrap extension: first BAND cols duplicated after col B
            for j in range(BAND // S):
                nc.sync.dma_start(
                    out=zf[m][:, B + S * j:B + S * (j + 1)],
                    in_=cc_o[128 * j:128 * (j + 1), :])
            rem = BAND % S
            if rem:
                j = BAND // S
                nc.sync.dma_start(
                    out=zf[m][:, B + S * j:B + S * j + rem],
                    in_=cc_o[128 * j:128 * j + 128, 0:rem])

        # --- pos row (fp32): pos_b = zn_img[:,b] . zn_txt[:,b] -----------
        prod = psb.tile([128, S], F32, tag="sq")
        nc.vector.tensor_mul(prod[:], zn["img"][:], zn["txt"][:])
        ppos = pps.tile([16, S], F32, tag="colps")
        nc.tensor.matmul(ppos[0:1, :], ones_col[:], prod[:], start=True,
                         stop=True)
        possb = apool.tile([1, S], F32, name="possb")
        nc.vector.tensor_copy(possb[:], ppos[0:1, :])
        nc.sync.dma_start(out=t_out["pos"][:, :], in_=possb[:])

        stats = apool.tile([128, 8], F32)

        # fences: one strided matmul touching every zf chunk pins all
        # subsequent (dynamically-offset) matmuls behind the zf loads on the
        # in-order PE queue.
        def _fence(zft):
            f = pps.tile([16, S], F32, tag="colps")
            zv = zft.rearrange("p (n c) -> p n c", c=TS)[:, :, 0:1]
            nc.tensor.matmul(f[0:1, 0:ZEXT // TS], onesb[:],
                             zv.rearrange("p n o -> p (n o)"),
                             start=True, stop=True)

        def _band_rc(m, k, esc, eoff):
            """Emit the cyclic-band work for row-chunk k of modality m.
            esc[:, eoff:eoff+BAND] receives exp(sim/t) (masked edges)."""
            lhs = znb[m][:, TS * k:TS * (k + 1)]
            for ci, (bo, wdt) in enumerate(BCHUNKS):
                ps = pps.tile([128, wdt], F32, tag="simps")
                for q in range(0, wdt, 512):
                    wq = min(512, wdt - q)
                    nc.tensor.matmul(
                        ps[:, q:q + wq], lhs,
                        zf[m][:, ds(v + TS * k + bo + q, wq)],
                        start=True, stop=True)
                if ci == 0:      # tile u=0: strict upper (drop diag + lower)
                    nc.vector.tensor_tensor(
                        out=ps[:, 0:TS], in0=ps[:, 0:TS],
                        in1=masks[:, 0:TS], op=add)
                if ci == 2:      # tile u=W_II: keep q<=p (incl diag)
                    nc.vector.tensor_tensor(
                        out=ps[:, TS:2 * TS], in0=ps[:, TS:2 * TS],
                        in1=masks[:, TS:2 * TS], op=add)
                nc.scalar.activation(esc[:, eoff + bo:eoff + bo + wdt],
                                     ps[:], Exp, scale=INV_T)

        def _colsum_batch(esc, eoff, escm, n_extra, extra_off):
            """Selector-matmul batch: band pieces 0..3 (+ n_extra IT pieces)
            each land on their own PSUM partition; returns the psum tile."""
            cp = pps.tile([16, 512], F32, tag="colps")
            npieces = 4 + n_extra
            # piece 0..2: full 512-col pieces of the band colsum span
            for p in range(3):
                nc.tensor.matmul(
                    cp[0:npieces, 0:512], sel[:, 16 * p:16 * p + npieces],
                    esc[:, eoff + 512 * p:eoff + 512 * (p + 1)],
                    start=(p == 0), stop=False)
            # piece 3: band cols 1536..1919 + masked tile u=U_B (1920..2047)
            nc.tensor.matmul(
                cp[0:npieces, 0:384], sel[:, 48:48 + npieces],
                esc[:, eoff + 1536:eoff + 1920], start=False, stop=False)
            nc.tensor.matmul(
                cp[0:npieces, 384:512], sel[:, 48:48 + npieces],
                escm[:], start=False, stop=(n_extra == 0))
            for q in range(n_extra):
                nc.tensor.matmul(
                    cp[0:npieces, 0:512],
                    sel[:, 16 * (4 + q):16 * (4 + q) + npieces],
                    esc[:, extra_off + 512 * q:extra_off + 512 * (q + 1)],
                    start=False, stop=(q == n_extra - 1))
            return cp

        # ---- img row-chunks: II band + IT full ---------------------------
        _fence(zf["img"])
        fence_txt_done = False
        for k in range(4):
            esc = escp.tile([128, BAND + B], BF16, tag="esci")
            _band_rc("img", k, esc, 0)
            if not fence_txt_done:
                _fence(zf["txt"])
                fence_txt_done = True
            for q in range(4):
                ps = pps.tile([128, 1024], F32, tag="simps")
                for h in range(2):
                    co = 1024 * q + 512 * h
                    nc.tensor.matmul(
                        ps[:, 512 * h:512 * (h + 1)],
                        znb["img"][:, TS * k:TS * (k + 1)],
                        zf["txt"][:, co:co + 512], start=True, stop=True)
                nc.scalar.activation(
                    esc[:, BAND + 1024 * q:BAND + 1024 * (q + 1)],
                    ps[:], Exp, scale=INV_T)
            escm = escp.tile([128, TS], BF16, tag="escm")
            nc.vector.tensor_tensor(out=escm[:], in0=esc[:, U_B * TS:CSPAN],
                                    in1=cmask[:], op=mult)
            nc.vector.tensor_reduce(out=stats[:, k:k + 1], in_=esc[:],
                                    axis=mybir.AxisListType.X, op=add)
            cp = _colsum_batch(esc, 0, escm, 8, BAND)
            cext = escp.tile([12, 512], F32, tag="cext")
            nc.vector.tensor_copy(cext[:], cp[0:12, :])
            nc.sync.dma_start(out=t_out["colII"][4 * k:4 * k + 4, :],
                              in_=cext[0:4, :])
            nc.sync.dma_start(out=t_out["colIT"][8 * k:8 * k + 8, :],
                              in_=cext[4:12, :])

        # ---- txt row-chunks: TT band only --------------------------------
        for k in range(4):
            esc = escp.tile([128, BAND], BF16, tag="esct")
            _band_rc("txt", k, esc, 0)
            escm = escp.tile([128, TS], BF16, tag="escm")
            nc.vector.tensor_tensor(out=escm[:], in0=esc[:, U_B * TS:CSPAN],
                                    in1=cmask[:], op=mult)
            nc.vector.tensor_reduce(out=stats[:, 4 + k:5 + k], in_=esc[:],
                                    axis=mybir.AxisListType.X, op=add)
            cp = _colsum_batch(esc, 0, escm, 0, 0)
            cext = escp.tile([4, 512], F32, tag="cextt")
            nc.vector.tensor_copy(cext[:], cp[0:4, :])
            nc.sync.dma_start(out=t_out["colTT"][4 * k:4 * k + 4, :],
                              in_=cext[0:4, :])

        nc.sync.dma_start(out=t_out["rows"][:, :], in_=stats[:])


def _prep_in_maps(inputs):
    host = {}
    for m in ("img", "txt"):
        host[f"We_{m}"] = np.ascontiguousarray(inputs[f"We_{m}"]).astype(NPBF)
        host[f"Wp1_{m}"] = np.ascontiguousarray(inputs[f"Wp1_{m}"]).astype(NPBF)
        host[f"Wp2_{m}"] = np.ascontiguousarray(inputs[f"Wp2_{m}"]).astype(NPBF)
        host[f"beT_{m}"] = np.ascontiguousarray(
            np.asarray(inputs[f"be_{m}"], np.float32).reshape(DE // 128, 128).T)
        host[f"bp1T_{m}"] = np.ascontiguousarray(
            np.asarray(inputs[f"bp1_{m}"], np.float32).reshape(DH // 128, 128).T)
        host[f"bp2T_{m}"] = np.ascontiguousarray(
            np.asarray(inputs[f"bp2_{m}"], np.float32).reshape(DP // 128, 128).T)
    p = np.arange(TS)[:, None]
    q = np.arange(TS)[None, :]
    mu = np.where(q > p, 0.0, NEG).astype(np.float32)        # u=0 edge
    ml = np.where(q <= p, 0.0, NEG).astype(np.float32)       # u=W_II edge
    host["masks"] = np.ascontiguousarray(np.concatenate([mu, ml], axis=1))
    host["cmask"] = np.ascontiguousarray((q < p).astype(NPBF))
    selm = np.zeros((128, 16, 16), np.float32)
    for j in range(16):
        selm[:, j, j] = 1.0
    host["sel"] = np.ascontiguousarray(selm.reshape(128, 256).astype(NPBF))

    x = {"img": np.asarray(inputs["x_image"], np.float32),
         "txt": np.asarray(inputs["x_text"], np.float32)}
    in_maps = []
    for c in range(NCORES):
        mp = dict(host)
        for m in ("img", "txt"):
            mp[f"xT_{m}"] = np.ascontiguousarray(
                x[m][c * S:(c + 1) * S].T).astype(NPBF)
        mp["voff"] = np.array([[c * S]], np.int32)
        in_maps.append(mp)
    return in_maps


def _finish_host(results):
    t = TEMP
    N = 2 * B
    rowsum = np.zeros(N)
    colsum = np.zeros(N)
    pos = np.zeros(N)
    for c in range(NCORES):
        r = np.asarray(results[c]["rows"], np.float64)
        cII = np.asarray(results[c]["colII"], np.float64)
        cIT = np.asarray(results[c]["colIT"], np.float64)
        cTT = np.asarray(results[c]["colTT"], np.float64)
        pc = np.asarray(results[c]["pos"], np.float64).ravel()
        pos[c * S:(c + 1) * S] = pc
        pos[B + c * S:B + (c + 1) * S] = pc
        for k in range(4):
            g0 = (4 * c + k) * TS
            rowsum[g0:g0 + TS] = r[:, k]
            rowsum[B + g0:B + g0 + TS] = r[:, 4 + k]
            idx = (g0 + np.arange(CSPAN)) % B
            np.add.at(colsum, idx, cII[4 * k:4 * k + 4].ravel())
            np.add.at(colsum, B + idx, cTT[4 * k:4 * k + 4].ravel())
        colsum[B:] += cIT.reshape(4, B).sum(axis=0)
    Tp = rowsum + colsum + np.exp(pos / t)
    loss = np.mean(np.log(Tp) - pos / t)
    return np.float32(loss)


def kernel(**inputs) -> np.ndarray:
    nc = _CACHE.get("nc")
    if nc is None:
        nc = _build()
        _CACHE["nc"] = nc
    res = run_bass_kernel_spmd(nc, _prep_in_maps(inputs),
                               core_ids=list(range(NCORES)))
    return _finish_host(res.results)


# revision 6
# speedup vs baseline: 1.0981x; 1.0981x over previous
"""CLIP-style contrastive (NT-Xent) loss on 8 Trainium2 NeuronCores.

v3: quadrant-band symmetric scheme — exploits sim-matrix symmetry to halve
the ScalarE exp work (the v1 bottleneck: 67us of EXP at full width).

  - Data-parallel batch shard (512 rows/core), towers in transposed
    activation layout (weights as stationary lhsT, bf16).
  - img*img and txt*txt quadrants are symmetric: each 128-row chunk
    computes only a cyclic band of 2176+128 columns (tiles u=0..17; u=0
    strict-upper, u=17 lower-incl-diag).  The mirror half of every row sum
    is recovered from column sums (tiles u=0..14 full, u=15 strict-lower).
    img*txt is computed once: row sums feed img rows, column sums feed txt
    rows.  Verified exact in fp64 (proto_quad.py).
  - Per-core band offsets (c*512) enter through one runtime register
    (reg_load of an int32 input) used as a dynamic ds() offset on the
    moving operand; the program stays SPMD-uniform.  A strided "fence"
    matmul pins dynamically-offset reads behind the zf loads.
  - Column sums: ones-matmuls with one-hot selector lhsT place each
    512-col piece on its own PSUM partition; one wide DVE copy extracts a
    whole row-chunk batch (DMA cannot read PSUM).
  - Row sums ride the Exp activations' accum_out (ScalarE), reduced at the
    end on DVE.
  - One dma_start per tensor (batched descriptors), spread across the
    sync/vector/scalar HWDGE queues: v2's 80 small DMAs kept the sync
    sequencer busy 58us and pushed the collectives out to t=87us.
  - A dummy 128-byte AllGather issues at t~0 to absorb the ~15us
    first-collective warmup on the CC core off the critical path.
  - Host finish (fp64): T'_g = rowsum_g + colsum_g + exp(pos_g/t);
    loss = mean(log T' - pos/t).  The diagonal is excluded on-device by
    the masks, so no giant-term cancellation anywhere.
"""

import os

os.environ.setdefault("NEURON_RT_DBG_RDH_CC", "0")

import numpy as np
import ml_dtypes

import concourse.bacc as bacc
import concourse.bass as bass
import concourse.mybir as mybir
import concourse.tile as tile
from concourse.bass import ds
from concourse.bass_utils import run_bass_kernel_spmd

NCORES = 8
B, DIN, DE, DH, DP = 4096, 1024, 512, 256, 128
S = B // NCORES            # 512 per-core batch shard
TS = 128
NBT = B // TS              # 32 column tiles per modality
W_II = 17                  # band tiles u=0..W_II (edges masked)
U_B = NBT - W_II           # 15: colsum tiles u=0..U_B (u=U_B strict-lower)
BAND = (W_II + 1) * TS     # 2304 band cols per row-chunk
CSPAN = (U_B + 1) * TS     # 2048 colsum span per band row-chunk
ZEXT = B + BAND            # 6400 zf cols incl wrap extension
TEMP = 0.07
INV_T = 1.0 / TEMP
NEG = -1.0e30

F32 = mybir.dt.float32
BF16 = mybir.dt.bfloat16
I32 = mybir.dt.int32
NPBF = ml_dtypes.bfloat16

BCHUNKS = ((0, 1536), (1536, 768))           # band: 2304 cols
ICHUNKS = ((0, 1536), (1536, 1536), (3072, 1024))   # IT: 4096 cols

_CACHE: dict = {}


def _build():
    nc = bacc.Bacc("TRN2", target_bir_lowering=False, debug=False,
                   num_devices=NCORES)

    t_in = {}
    for m in ("img", "txt"):
        t_in[f"xT_{m}"] = nc.dram_tensor(f"xT_{m}", [DIN, S], BF16,
                                         kind="ExternalInput")
        t_in[f"We_{m}"] = nc.dram_tensor(f"We_{m}", [DIN, DE], BF16,
                                         kind="ExternalInput")
        t_in[f"Wp1_{m}"] = nc.dram_tensor(f"Wp1_{m}", [DE, DH], BF16,
                                          kind="ExternalInput")
        t_in[f"Wp2_{m}"] = nc.dram_tensor(f"Wp2_{m}", [DH, DP], BF16,
                                          kind="ExternalInput")
        t_in[f"beT_{m}"] = nc.dram_tensor(f"beT_{m}", [128, DE // 128], F32,
                                          kind="ExternalInput")
        t_in[f"bp1T_{m}"] = nc.dram_tensor(f"bp1T_{m}", [128, DH // 128], F32,
                                           kind="ExternalInput")
        t_in[f"bp2T_{m}"] = nc.dram_tensor(f"bp2T_{m}", [128, DP // 128], F32,
                                           kind="ExternalInput")
    t_in["masks"] = nc.dram_tensor("masks", [128, 2 * TS], F32,
                                   kind="ExternalInput")
    t_in["cmask"] = nc.dram_tensor("cmask", [128, TS], BF16,
                                   kind="ExternalInput")
    t_in["sel"] = nc.dram_tensor("sel", [128, 16 * 16], BF16,
                                 kind="ExternalInput")
    t_in["voff"] = nc.dram_tensor("voff", [1, 1], I32, kind="ExternalInput")

    t_out = {
        "rows": nc.dram_tensor("rows", [128, 8], F32, kind="ExternalOutput"),
        "pos": nc.dram_tensor("pos", [1, S], F32, kind="ExternalOutput"),
        "colII": nc.dram_tensor("colII", [16, 512], F32,
                                kind="ExternalOutput"),
        "colIT": nc.dram_tensor("colIT", [32, 512], F32,
                                kind="ExternalOutput"),
        "colTT": nc.dram_tensor("colTT", [16, 512], F32,
                                kind="ExternalOutput"),
    }

    with tile.TileContext(nc) as tc:
        _emit(nc, tc, t_in, t_out)
    nc.compile()
    return nc


def _load_weights(nc, wpool, t_in, m, eng):
    """One batched dma_start per tensor, on the given engine's DGE queue."""
    xt = wpool.tile([128, (DIN // 128) * S], BF16, name=f"xt_{m}")
    we = wpool.tile([128, (DIN // 128) * DE], BF16, name=f"we_{m}")
    eng.dma_start(
        out=we[:].rearrange("p (k d) -> p k d", k=DIN // 128),
        in_=t_in[f"We_{m}"][:, :].rearrange("(k p) d -> p k d", p=128))
    eng.dma_start(
        out=xt[:].rearrange("p (k d) -> p k d", k=DIN // 128),
        in_=t_in[f"xT_{m}"][:, :].rearrange("(k p) d -> p k d", p=128))
    wp1 = wpool.tile([128, (DE // 128) * DH], BF16, name=f"wp1_{m}")
    eng.dma_start(
        out=wp1[:].rearrange("p (k d) -> p k d", k=DE // 128),
        in_=t_in[f"Wp1_{m}"][:, :].rearrange("(k p) d -> p k d", p=128))
    wp2 = wpool.tile([128, (DH // 128) * DP], BF16, name=f"wp2_{m}")
    eng.dma_start(
        out=wp2[:].rearrange("p (k d) -> p k d", k=DH // 128),
        in_=t_in[f"Wp2_{m}"][:, :].rearrange("(k p) d -> p k d", p=128))
    beT = wpool.tile([128, DE // 128], F32, name=f"beT_{m}")
    eng.dma_start(out=beT[:], in_=t_in[f"beT_{m}"][:, :])
    bp1T = wpool.tile([128, DH // 128], F32, name=f"bp1T_{m}")
    eng.dma_start(out=bp1T[:], in_=t_in[f"bp1T_{m}"][:, :])
    bp2T = wpool.tile([128, DP // 128], F32, name=f"bp2T_{m}")
    eng.dma_start(out=bp2T[:], in_=t_in[f"bp2T_{m}"][:, :])
    return dict(xt=xt, we=we, wp1=wp1, wp2=wp2, beT=beT, bp1T=bp1T, bp2T=bp2T)


def _project(nc, pps, psb, w):
    """One tower's matmul stream (kept dense to stay HAM-warm).
    Returns the pre-normalization z tile (fp32 SBUF)."""
    add = mybir.AluOpType.add
    mx = mybir.AluOpType.max

    h = psb.tile([128, (DE // 128) * S], BF16, tag="h")
    for mm in range(DE // 128):
        ph = pps.tile([128, S], F32, tag="simps")
        for k in range(DIN // 128):
            nc.tensor.matmul(
                ph[:],
                w["we"][:, k * DE + 128 * mm: k * DE + 128 * (mm + 1)],
                w["xt"][:, k * S:(k + 1) * S],
                start=(k == 0), stop=(k == DIN // 128 - 1))
        nc.vector.tensor_scalar(
            out=h[:, mm * S:(mm + 1) * S], in0=ph[:],
            scalar1=w["beT"][:, mm:mm + 1], scalar2=None, op0=add)
    g = psb.tile([128, (DH // 128) * S], BF16, tag="g")
    for mm in range(DH // 128):
        pg = pps.tile([128, S], F32, tag="simps")
        for k in range(DE // 128):
            nc.tensor.matmul(
                pg[:],
                w["wp1"][:, k * DH + 128 * mm: k * DH + 128 * (mm + 1)],
                h[:, k * S:(k + 1) * S],
                start=(k == 0), stop=(k == DE // 128 - 1))
        nc.vector.tensor_scalar(
            out=g[:, mm * S:(mm + 1) * S], in0=pg[:],
            scalar1=w["bp1T"][:, mm:mm + 1], scalar2=0.0, op0=add, op1=mx)
    pz = pps.tile([128, S], F32, tag="simps")
    for k in range(DH // 128):
        nc.tensor.matmul(pz[:], w["wp2"][:, k * DP: k * DP + 128],
                         g[:, k * S:(k + 1) * S],
                         start=(k == 0), stop=(k == DH // 128 - 1))
    z = psb.tile([128, S], F32, tag="z")
    nc.vector.tensor_scalar(out=z[:], in0=pz[:], scalar1=w["bp2T"][:, 0:1],
                            scalar2=None, op0=add)
    return z


def _normalize(nc, pps, psb, apool, z, m, ones_col, ones_rowb):
    """inv = exp(-0.5*ln(|z|^2)) per column; returns (zn f32, znb bf16)."""
    Exp = mybir.ActivationFunctionType.Exp
    Ln = mybir.ActivationFunctionType.Ln
    sq = psb.tile([128, S], F32, tag="sq")
    nc.vector.tensor_mul(sq[:], z[:], z[:])
    pssq = pps.tile([16, S], F32, tag="colps")
    nc.tensor.matmul(pssq[0:1, :], ones_col[:], sq[:], start=True, stop=True)
    lnr = psb.tile([1, S], F32, tag="lnr")
    nc.scalar.activation(lnr[:], pssq[0:1, :], Ln)
    inv = psb.tile([1, S], BF16, tag="inv")
    nc.scalar.activation(inv[:], lnr[:], Exp, scale=-0.5)
    pinvb = pps.tile([128, S], F32, tag="simps")
    nc.tensor.matmul(pinvb[:], ones_rowb[:], inv[:], start=True, stop=True)
    zn = apool.tile([128, S], F32, name=f"zn_{m}")
    nc.vector.tensor_mul(zn[:], z[:], pinvb[:])
    znb = apool.tile([128, S], BF16, name=f"znb_{m}")
    nc.vector.tensor_copy(znb[:], zn[:])
    return zn, znb


def _emit(nc, tc, t_in, t_out):
    Exp = mybir.ActivationFunctionType.Exp
    add = mybir.AluOpType.add
    mult = mybir.AluOpType.mult

    with tc.tile_pool(name="const", bufs=1) as cpool, \
         tc.tile_pool(name="wpool", bufs=1) as wpool, \
         tc.tile_pool(name="actpool", bufs=1) as apool, \
         tc.tile_pool(name="projsb", bufs=2) as psb, \
         tc.tile_pool(name="escp", bufs=2) as escp, \
         tc.tile_pool(name="psum", bufs=2, space="PSUM") as pps, \
         tc.tile_pool(name="dram", bufs=1, space="DRAM") as dram:

        # --- dummy collective: absorb CC first-collective warmup at t~0 ---
        warm = cpool.tile([128, 1], BF16)
        nc.any.memset(warm[:], 0.0)
        warm_d = dram.tile([128, 1], BF16, name="warm_d")
        nc.scalar.dma_start(out=warm_d[:, :], in_=warm[:])
        warm_o = dram.tile([128 * NCORES, 1], BF16, name="warm_o",
                           addr_space="Shared")
        nc.gpsimd.collective_compute(
            "AllGather", mybir.AluOpType.bypass,
            replica_groups=[list(range(NCORES))],
            ins=[warm_d[:]], outs=[warm_o[:]])

        # --- small constants / per-core offset register (scalar queue) ----
        vofft = wpool.tile([1, 1], I32, name="vofft")
        nc.scalar.dma_start(out=vofft[:], in_=t_in["voff"][:, :])
        masks = wpool.tile([128, 2 * TS], F32, name="masks")
        nc.scalar.dma_start(out=masks[:], in_=t_in["masks"][:, :])
        cmask = wpool.tile([128, TS], BF16, name="cmask")
        nc.scalar.dma_start(out=cmask[:], in_=t_in["cmask"][:, :])
        sel = wpool.tile([128, 16 * 16], BF16, name="sel")
        nc.scalar.dma_start(out=sel[:], in_=t_in["sel"][:, :])

        reg = nc.tensor.alloc_register("voff_reg")
        nc.reg_load(reg, vofft[0:1, 0:1])
        v = nc.snap(reg, min_val=0, max_val=(NCORES - 1) * S)

        ones_col = cpool.tile([128, 1], F32)
        nc.any.memset(ones_col[:], 1.0)
        ones_rowb = cpool.tile([1, 128], BF16)
        nc.any.memset(ones_rowb[:], 1.0)
        onesb = cpool.tile([128, 1], BF16)
        nc.any.memset(onesb[:], 1.0)

        # --- tower operands: img on sync queue, txt on vector queue -------
        w_all = {"img": _load_weights(nc, wpool, t_in, "img", nc.sync),
                 "txt": _load_weights(nc, wpool, t_in, "txt", nc.scalar)}

        zn, znb, zf = {}, {}, {}
        for m in ("img", "txt"):
            zf[m] = apool.tile([128, ZEXT], BF16, name=f"zf_{m}")

        def _ag(m):
            cc_in = dram.tile([128, S], BF16, name=f"cc_in_{m}")
            nc.scalar.dma_start(out=cc_in[:, :], in_=znb[m][:])
            cc_o = dram.tile([128 * NCORES, S], BF16, name=f"cc_out_{m}",
                             addr_space="Shared")
            nc.gpsimd.collective_compute(
                "AllGather", mybir.AluOpType.bypass,
                replica_groups=[list(range(NCORES))],
                ins=[cc_in[:]], outs=[cc_o[:]])
            nc.sync.dma_start(
                out=zf[m][:, 0:B].rearrange("p (j d) -> p j d", j=NCORES),
                in_=cc_o[:].rearrange("(j p) d -> p j d", p=128))
            nex = BAND // S
            nc.sync.dma_start(
                out=zf[m][:, B:B + nex * S].rearrange(
                    "p (j d) -> p j d", j=nex),
                in_=cc_o[0:nex * 128, :].rearrange("(j p) d -> p j d", p=128))
            rem = BAND % S
            if rem:
                nc.sync.dma_start(
                    out=zf[m][:, B + nex * S:B + BAND],
                    in_=cc_o[128 * nex:128 * (nex + 1), 0:rem])

        # img tower -> normalize -> AG (overlaps txt tower)
        z_i = _project(nc, pps, psb, w_all["img"])
        zn["img"], znb["img"] = _normalize(nc, pps, psb, apool, z_i, "img",
                                           ones_col, ones_rowb)
        _ag("img")
        z_t = _project(nc, pps, psb, w_all["txt"])
        zn["txt"], znb["txt"] = _normalize(nc, pps, psb, apool, z_t, "txt",
                                           ones_col, ones_rowb)
        _ag("txt")

        # pos row (fp32): pos_b = zn_img[:,b] . zn_txt[:,b]
        prod = psb.tile([128, S], F32, tag="sq")
        nc.vector.tensor_mul(prod[:], zn["img"][:], zn["txt"][:])
        ppos = pps.tile([16, S], F32, tag="colps")
        nc.tensor.matmul(ppos[0:1, :], ones_col[:], prod[:], start=True,
                         stop=True)
        possb = apool.tile([1, S], F32, name="possb")
        nc.vector.tensor_copy(possb[:], ppos[0:1, :])
        nc.sync.dma_start(out=t_out["pos"][:, :], in_=possb[:])

        # rowsum partials via Exp accum_out: img rc: 2 band + 3 IT; txt: 2
        statsI = apool.tile([128, 4 * 5], F32)
        statsT = apool.tile([128, 4 * 2], F32)

        def _fence(zft):
            f = pps.tile([16, S], F32, tag="colps")
            zv = zft.rearrange("p (n c) -> p n c", c=TS)[:, :, 0:1]
            nc.tensor.matmul(f[0:1, 0:ZEXT // TS], onesb[:],
                             zv.rearrange("p n o -> p (n o)"),
                             start=True, stop=True)

        def _band_rc(m, k, esc, stat, soff):
            lhs = znb[m][:, TS * k:TS * (k + 1)]
            for ci, (bo, wdt) in enumerate(BCHUNKS):
                ps = pps.tile([128, wdt], F32, tag="simps")
                for q in range(0, wdt, 512):
                    wq = min(512, wdt - q)
                    nc.tensor.matmul(
                        ps[:, q:q + wq], lhs,
                        zf[m][:, ds(v + TS * k + bo + q, wq)],
                        start=True, stop=True)
                if ci == 0:      # tile u=0: strict upper
                    nc.vector.tensor_tensor(
                        out=ps[:, 0:TS], in0=ps[:, 0:TS],
                        in1=masks[:, 0:TS], op=add)
                else:            # tile u=W_II at chunk offset 640
                    nc.vector.tensor_tensor(
                        out=ps[:, 640:768], in0=ps[:, 640:768],
                        in1=masks[:, TS:2 * TS], op=add)
                nc.scalar.activation(
                    esc[:, bo:bo + wdt], ps[:], Exp, scale=INV_T,
                    accum_out=stat[:, soff + ci:soff + ci + 1])

        def _colsum_batch(esc, escm, n_extra, extra_off):
            cp = pps.tile([16, 512], F32, tag="colps")
            npieces = 4 + n_extra
            for p in range(3):
                nc.tensor.matmul(
                    cp[0:npieces, 0:512], sel[:, 16 * p:16 * p + npieces],
                    esc[:, 512 * p:512 * (p + 1)],
                    start=(p == 0), stop=False)
            nc.tensor.matmul(
                cp[0:npieces, 0:384], sel[:, 48:48 + npieces],
                esc[:, 1536:1920], start=False, stop=False)
            nc.tensor.matmul(
                cp[0:npieces, 384:512], sel[:, 48:48 + npieces],
                escm[:], start=False, stop=(n_extra == 0))
            for q in range(n_extra):
                nc.tensor.matmul(
                    cp[0:npieces, 0:512],
                    sel[:, 16 * (4 + q):16 * (4 + q) + npieces],
                    esc[:, extra_off + 512 * q:extra_off + 512 * (q + 1)],
                    start=False, stop=(q == n_extra - 1))
            return cp

        # ---- img row-chunks: II band + IT full ---------------------------
        _fence(zf["img"])
        fence_txt_done = False
        for k in range(4):
            esc = escp.tile([128, BAND + B], BF16, tag="esci")
            _band_rc("img", k, esc, statsI, 5 * k)
            if not fence_txt_done:
                _fence(zf["txt"])
                fence_txt_done = True
            for ci, (co, wdt) in enumerate(ICHUNKS):
                ps = pps.tile([128, wdt], F32, tag="simps")
                for q in range(0, wdt, 512):
                    nc.tensor.matmul(
                        ps[:, q:q + 512],
                        znb["img"][:, TS * k:TS * (k + 1)],
                        zf["txt"][:, co + q:co + q + 512],
                        start=True, stop=True)
                nc.scalar.activation(
                    esc[:, BAND + co:BAND + co + wdt], ps[:], Exp,
                    scale=INV_T,
                    accum_out=statsI[:, 5 * k + 2 + ci:5 * k + 3 + ci])
            escm = escp.tile([128, TS], BF16, tag="escm")
            nc.vector.tensor_tensor(out=escm[:], in0=esc[:, U_B * TS:CSPAN],
                                    in1=cmask[:], op=mult)
            cp = _colsum_batch(esc, escm, 8, BAND)
            cext = escp.tile([12, 512], F32, tag="cext")
            nc.vector.tensor_copy(cext[:], cp[0:12, :])
            nc.sync.dma_start(out=t_out["colII"][4 * k:4 * k + 4, :],
                              in_=cext[0:4, :])
            nc.sync.dma_start(out=t_out["colIT"][8 * k:8 * k + 8, :],
                              in_=cext[4:12, :])

        # ---- txt row-chunks: TT band only --------------------------------
        for k in range(4):
            esc = escp.tile([128, BAND], BF16, tag="esct")
            _band_rc("txt", k, esc, statsT, 2 * k)
            escm = escp.tile([128, TS], BF16, tag="escm")
            nc.vector.tensor_tensor(out=escm[:], in0=esc[:, U_B * TS:CSPAN],
                                    in1=cmask[:], op=mult)
            cp = _colsum_batch(esc, escm, 0, 0)
            cext = escp.tile([4, 512], F32, tag="cextt")
            nc.vector.tensor_copy(cext[:], cp[0:4, :])
            nc.sync.dma_start(out=t_out["colTT"][4 * k:4 * k + 4, :],
                              in_=cext[0:4, :])

        # ---- final row sums ---------------------------------------------
        outv = apool.tile([128, 8], F32)
        nc.vector.tensor_reduce(
            out=outv[:, 0:4],
            in_=statsI[:].rearrange("p (r t) -> p r t", t=5),
            axis=mybir.AxisListType.X, op=add)
        nc.vector.tensor_reduce(
            out=outv[:, 4:8],
            in_=statsT[:].rearrange("p (r t) -> p r t", t=2),
            axis=mybir.AxisListType.X, op=add)
        nc.sync.dma_start(out=t_out["rows"][:, :], in_=outv[:])


def _prep_in_maps(inputs):
    host = {}
    for m in ("img", "txt"):
        host[f"We_{m}"] = np.ascontiguousarray(inputs[f"We_{m}"]).astype(NPBF)
        host[f"Wp1_{m}"] = np.ascontiguousarray(inputs[f"Wp1_{m}"]).astype(NPBF)
        host[f"Wp2_{m}"] = np.ascontiguousarray(inputs[f"Wp2_{m}"]).astype(NPBF)
        host[f"beT_{m}"] = np.ascontiguousarray(
            np.asarray(inputs[f"be_{m}"], np.float32).reshape(DE // 128, 128).T)
        host[f"bp1T_{m}"] = np.ascontiguousarray(
            np.asarray(inputs[f"bp1_{m}"], np.float32).reshape(DH // 128, 128).T)
        host[f"bp2T_{m}"] = np.ascontiguousarray(
            np.asarray(inputs[f"bp2_{m}"], np.float32).reshape(DP // 128, 128).T)
    p = np.arange(TS)[:, None]
    q = np.arange(TS)[None, :]
    mu = np.where(q > p, 0.0, NEG).astype(np.float32)        # u=0 edge
    ml = np.where(q <= p, 0.0, NEG).astype(np.float32)       # u=W_II edge
    host["masks"] = np.ascontiguousarray(np.concatenate([mu, ml], axis=1))
    host["cmask"] = np.ascontiguousarray((q < p).astype(NPBF))
    selm = np.zeros((128, 16, 16), np.float32)
    for j in range(16):
        selm[:, j, j] = 1.0
    host["sel"] = np.ascontiguousarray(selm.reshape(128, 256).astype(NPBF))

    x = {"img": np.asarray(inputs["x_image"], np.float32),
         "txt": np.asarray(inputs["x_text"], np.float32)}
    in_maps = []
    for c in range(NCORES):
        mp = dict(host)
        for m in ("img", "txt"):
            mp[f"xT_{m}"] = np.ascontiguousarray(
                x[m][c * S:(c + 1) * S].T).astype(NPBF)
        mp["voff"] = np.array([[c * S]], np.int32)
        in_maps.append(mp)
    return in_maps


def _finish_host(results):
    t = TEMP
    N = 2 * B
    rowsum = np.zeros(N)
    colsum = np.zeros(N)
    pos = np.zeros(N)
    for c in range(NCORES):
        r = np.asarray(results[c]["rows"], np.float64)
        cII = np.asarray(results[c]["colII"], np.float64)
        cIT = np.asarray(results[c]["colIT"], np.float64)
        cTT = np.asarray(results[c]["colTT"], np.float64)
        pc = np.asarray(results[c]["pos"], np.float64).ravel()
        pos[c * S:(c + 1) * S] = pc
        pos[B + c * S:B + (c + 1) * S] = pc
        for k in range(4):
            g0 = (4 * c + k) * TS
            rowsum[g0:g0 + TS] = r[:, k]
            rowsum[B + g0:B + g0 + TS] = r[:, 4 + k]
            idx = (g0 + np.arange(CSPAN)) % B
            np.add.at(colsum, idx, cII[4 * k:4 * k + 4].ravel())
            np.add.at(colsum, B + idx, cTT[4 * k:4 * k + 4].ravel())
        colsum[B:] += cIT.reshape(4, B).sum(axis=0)
    Tp = rowsum + colsum + np.exp(pos / t)
    loss = np.mean(np.log(Tp) - pos / t)
    return np.float32(loss)


def kernel(**inputs) -> np.ndarray:
    nc = _CACHE.get("nc")
    if nc is None:
        nc = _build()
        _CACHE["nc"] = nc
    res = run_bass_kernel_spmd(nc, _prep_in_maps(inputs),
                               core_ids=list(range(NCORES)))
    return _finish_host(res.results)
